# revision 12
# baseline (speedup 1.0000x reference)
"""AdaptiveGCN Trainium2 kernel — 8-core pure data parallelism (1 batch elem/core).

Math notes (exact identities, valid for any inputs):
  - denom = 1 + sum_j softmax(...)_ij = 2 (softmax rows sum to 1), and a
    positive per-row scale cancels through relu+LayerNorm, so the /denom in
    the reference is a no-op on the final output.
  - mask = (rowsum + colsum == 0) is always False: each softmax row sums
    to 1 > 0.  So mask is computed host-side as all-False.

Per-core pipeline (L=128 tokens, D=768):
  hiT/hjT = (x @ w1[:D]).T, (x @ w1[D:]).T  (d-on-partitions, bf16)
  edge MLP over 32 groups of 4 rows i:
      t_k = relu(hjT_k + hiT_k[:, i])          (DVE tensor_scalar, bf16)
      o2T = w2.T @ t                            (PE, PSUM f32, N=512)
      r   = relu(o2T + b2)                      (ACT, PSUM->SBUF bf16)
      ew[i, :] = w3.T @ r                       (PE, rows of a PSUM tile)
  ew = sigmoid(ew + b3); A = softmax(adj*ew + I) row-wise; aT via PE transpose
  3 GCN layers: ax = A@out; z = (ax+out)@gw + 2gb; out = LN(relu(z))*lng+lnb
"""

import sys

if "/opt/trn_rl_repo" not in sys.path:
    sys.path.insert(0, "/opt/trn_rl_repo")

import numpy as np
import ml_dtypes

BF = ml_dtypes.bfloat16
B, L, D = 8, 128, 768
KC = D // 128      # 6 contraction chunks
NC = (D // 2) // 128  # 3 hidden chunks
IB = 4             # rows per edge-MLP group
NG = L // IB       # 32 groups

_cache = {}


def _build_bass():
    import concourse.tile as tile
    from concourse import bacc, mybir

    f32 = mybir.dt.float32
    bf16 = mybir.dt.bfloat16
    AF = mybir.ActivationFunctionType
    OP = mybir.AluOpType

    nc = bacc.Bacc("TRN2", target_bir_lowering=False, debug=False, num_devices=8)

    def din(name, shape, dt):
        return nc.dram_tensor(name, shape, dt, kind="ExternalInput").ap()

    xb_d = din("xb", (L, D), bf16)
    xTb_d = din("xTb", (D, L), bf16)
    adjT_d = din("adjT", (L, L), f32)
    eye_d = din("eye", (L, L), f32)
    onescol_d = din("onescol", (L, 1), bf16)
    onef_d = din("onef", (1, 1), f32)
    idb_d = din("idb", (L, L), bf16)
    w1b_d = din("w1b", (2 * D, D), bf16)
    w2b_d = din("w2b", (D, D // 2), bf16)
    w3b_d = din("w3b", (D // 2, 1), bf16)
    b1c_d = din("b1c", (D, 1), f32)
    b2c_d = din("b2c", (D // 2, 1), f32)
    b3bc_d = din("b3bc", (L, 1), f32)
    epsc_d = din("epsc", (L, 1), f32)
    gwb_d = din("gwb", (3, D, D), bf16)
    gb2b_d = din("gb2b", (3, 1, D), bf16)
    onesb_d = din("onesb", (1, L), bf16)
    lngb_d = din("lngb", (3, L, D), bf16)
    lnbb_d = din("lnbb", (3, L, D), bf16)
    out_d = nc.dram_tensor("out", (L, D), f32, kind="ExternalOutput").ap()

    with tile.TileContext(nc) as tc:
        with tc.tile_pool(name="const", bufs=1) as cp:
            # ---- load constants ----
            xb = cp.tile([L, D], bf16)
            nc.sync.dma_start(xb[:], xb_d[:])
            xT = cp.tile([128, KC * 128], bf16)
            for k in range(KC):
                nc.sync.dma_start(xT[:, k * 128:(k + 1) * 128],
                                  xTb_d[k * 128:(k + 1) * 128, :])
            adjT_s = cp.tile([L, L], f32)
            nc.sync.dma_start(adjT_s[:], adjT_d[:])
            eye_s = cp.tile([L, L], f32)
            nc.sync.dma_start(eye_s[:], eye_d[:])
            onescol_s = cp.tile([L, 1], bf16)
            nc.sync.dma_start(onescol_s[:], onescol_d[:])
            onef_s = cp.tile([1, 1], f32)
            nc.sync.dma_start(onef_s[:], onef_d[:])
            idb_s = cp.tile([L, L], bf16)
            nc.sync.dma_start(idb_s[:], idb_d[:])
            w1_s = cp.tile([128, 12 * D], bf16)
            for r in range(12):
                nc.sync.dma_start(w1_s[:, r * D:(r + 1) * D],
                                  w1b_d[r * 128:(r + 1) * 128, :])
            w2_s = cp.tile([128, KC * (D // 2)], bf16)
            for k in range(KC):
                nc.sync.dma_start(w2_s[:, k * 384:(k + 1) * 384],
                                  w2b_d[k * 128:(k + 1) * 128, :])
            w3_s = cp.tile([128, NC], bf16)
            for n in range(NC):
                nc.sync.dma_start(w3_s[:, n:n + 1],
                                  w3b_d[n * 128:(n + 1) * 128, :])
            b1c_s = cp.tile([128, KC], f32)
            for k in range(KC):
                nc.sync.dma_start(b1c_s[:, k:k + 1],
                                  b1c_d[k * 128:(k + 1) * 128, :])
            b2c_s = cp.tile([128, NC], f32)
            for n in range(NC):
                nc.sync.dma_start(b2c_s[:, n:n + 1],
                                  b2c_d[n * 128:(n + 1) * 128, :])
            b3bc_s = cp.tile([L, 1], f32)
            nc.sync.dma_start(b3bc_s[:], b3bc_d[:])
            epsc_s = cp.tile([L, 1], f32)
            nc.sync.dma_start(epsc_s[:], epsc_d[:])
            gw_s = cp.tile([128, 3 * KC * D], bf16)
            for li in range(3):
                for k in range(KC):
                    nc.sync.dma_start(
                        gw_s[:, (li * KC + k) * D:(li * KC + k + 1) * D],
                        gwb_d[li, k * 128:(k + 1) * 128, :])
            gb2_s = cp.tile([1, 3 * D], bf16)
            for li in range(3):
                nc.sync.dma_start(gb2_s[:, li * D:(li + 1) * D], gb2b_d[li])
            ones_s = cp.tile([1, L], bf16)
            nc.sync.dma_start(ones_s[:], onesb_d[:])
            lng_s = cp.tile([L, 3 * D], bf16)
            lnb_s = cp.tile([L, 3 * D], bf16)
            for li in range(3):
                nc.sync.dma_start(lng_s[:, li * D:(li + 1) * D], lngb_d[li])
                nc.sync.dma_start(lnb_s[:, li * D:(li + 1) * D], lnbb_d[li])

            hiT = cp.tile([128, D], f32)   # chunk m at cols m*128, +b1 folded
            hjT = cp.tile([128, D], bf16)

            # ---- hiT / hjT ----
            with tc.tile_pool(name="phi", bufs=2, space="PSUM") as phi:
                for half, dst, with_bias in ((0, hiT, True), (1, hjT, False)):
                    for m in range(KC):
                        ph = phi.tile([128, 128], f32)
                        for k in range(KC):
                            nc.tensor.matmul(
                                ph[:],
                                w1_s[:, (half * KC + k) * D + m * 128:
                                     (half * KC + k) * D + (m + 1) * 128],
                                xT[:, k * 128:(k + 1) * 128],
                                start=(k == 0), stop=(k == KC - 1))
                        if with_bias:
                            nc.vector.tensor_scalar(
                                dst[:, m * 128:(m + 1) * 128], ph[:],
                                b1c_s[:, m:m + 1], None, OP.add)
                        else:
                            nc.scalar.copy(dst[:, m * 128:(m + 1) * 128], ph[:])

            # ---- edge MLP ----
            ew_sb = cp.tile([L, L], f32)
            with tc.tile_pool(name="tp", bufs=2) as tp, \
                 tc.tile_pool(name="rp", bufs=2) as rp, \
                 tc.tile_pool(name="po2", bufs=2, space="PSUM") as po2, \
                 tc.tile_pool(name="pew", bufs=1, space="PSUM") as pewp:
                pew = pewp.tile([L, L], f32)
                for g in range(NG):
                    ts = []
                    for k in range(KC):
                        t = tp.tile([128, IB * 128], bf16, tag=f"t{k}")
                        for s in range(IB):
                            i = g * IB + s
                            nc.vector.tensor_scalar(
                                t[:, s * 128:(s + 1) * 128],
                                hjT[:, k * 128:(k + 1) * 128],
                                hiT[:, k * 128 + i:k * 128 + i + 1],
                                0.0, OP.add, OP.max)
                        ts.append(t)
                    rs_ = []
                    for n in range(NC):
                        po = po2.tile([128, IB * 128], f32)
                        for k in range(KC):
                            nc.tensor.matmul(
                                po[:],
                                w2_s[:, k * 384 + n * 128:k * 384 + (n + 1) * 128],
                                ts[k][:],
                                start=(k == 0), stop=(k == KC - 1))
                        r = rp.tile([128, IB * 128], bf16, tag=f"r{n}")
                        nc.scalar.activation(r[:], po[:], AF.Relu,
                                             bias=b2c_s[:, n:n + 1])
                        rs_.append(r)
                    for s in range(IB):
                        i = g * IB + s
                        for n in range(NC):
                            nc.tensor.matmul(
                                pew[:, i:i + 1],
                                rs_[n][:, s * 128:(s + 1) * 128],
                                w3_s[:, n:n + 1],
                                start=(n == 0), stop=(n == NC - 1))
                # ewT (j on partitions, i free)
                nc.scalar.activation(ew_sb[:], pew[:], AF.Sigmoid,
                                     bias=b3bc_s[:, 0:1])

            # ---- transposed softmax: aT_unnorm = exp(adjT*ewT + I), rs = 1/colsum
            a0 = cp.tile([L, L], f32)
            nc.vector.tensor_mul(a0[:], adjT_s[:], ew_sb[:])
            a1 = cp.tile([L, L], f32)
            nc.vector.tensor_add(a1[:], a0[:], eye_s[:])
            expT_bf = cp.tile([L, L], bf16)
            nc.scalar.activation(expT_bf[:], a1[:], AF.Exp)

            with tc.tile_pool(name="gp", bufs=2) as gp, \
                 tc.tile_pool(name="pax", bufs=1, space="PSUM") as pax, \
                 tc.tile_pool(name="ptp", bufs=2, space="PSUM") as ptp, \
                 tc.tile_pool(name="ptr", bufs=1, space="PSUM") as ptr:
                pes = ptr.tile([1, L], f32, tag="pes")
                nc.tensor.matmul(pes[:], onescol_s[:], expT_bf[:],
                                 start=True, stop=True)
                esr = cp.tile([1, L], f32)
                nc.scalar.copy(esr[:], pes[:])
                pesT = ptr.tile([L, 1], f32, tag="pesT")
                nc.tensor.matmul(pesT[:], esr[:], onef_s[:],
                                 start=True, stop=True)
                rs_col = cp.tile([L, 1], f32)
                nc.vector.reciprocal(rs_col[:], pesT[:])

                out_cur = xb
                for li in range(3):
                    px = pax.tile([L, D], f32, tag="pax")
                    nc.tensor.matmul(px[:, 0:512], expT_bf[:], out_cur[:, 0:512],
                                     start=True, stop=True)
                    nc.tensor.matmul(px[:, 512:D], expT_bf[:], out_cur[:, 512:D],
                                     start=True, stop=True)
                    s_nat = gp.tile([L, D], bf16, tag="s_nat")
                    nc.vector.scalar_tensor_tensor(
                        s_nat[:], px[:], rs_col[:, 0:1], out_cur[:],
                        OP.mult, OP.add)
                    sT = gp.tile([128, D], bf16, tag="sT")
                    for k in range(KC):
                        pt = ptp.tile([128, 128], bf16, tag="pt")
                        nc.tensor.transpose(pt[:], s_nat[:, k * 128:(k + 1) * 128],
                                            idb_s[:])
                        nc.scalar.copy(sT[:, k * 128:(k + 1) * 128], pt[:])
                    pz = pax.tile([L, D], f32, tag="pz")
                    for c0, c1 in ((0, 512), (512, D)):
                        for k in range(KC):
                            nc.tensor.matmul(
                                pz[:, c0:c1],
                                sT[:, k * 128:(k + 1) * 128],
                                gw_s[:, (li * KC + k) * D + c0:(li * KC + k) * D + c1],
                                start=(k == 0), stop=False)
                        nc.tensor.matmul(pz[:, c0:c1], ones_s[:],
                                         gb2_s[:, li * D + c0:li * D + c1],
                                         start=False, stop=True)
                    zr = gp.tile([L, D], f32, tag="zr")
                    sm = gp.tile([L, 1], f32, tag="sm")
                    nc.scalar.activation(zr[:], pz[:], AF.Relu, accum_out=sm[:])
                    scr = gp.tile([L, D], f32, tag="scr")
                    s2 = gp.tile([L, 1], f32, tag="s2")
                    nc.scalar.activation(scr[:], zr[:], AF.Square, accum_out=s2[:])
                    nm = gp.tile([L, 1], f32, tag="nm")
                    nc.vector.tensor_scalar_mul(nm[:], sm[:], -1.0 / D)
                    msq = gp.tile([L, 1], f32, tag="msq")
                    nc.vector.tensor_mul(msq[:], nm[:], nm[:])
                    var = gp.tile([L, 1], f32, tag="var")
                    nc.vector.tensor_scalar(var[:], s2[:], 1.0 / D, msq[:, 0:1],
                                            OP.mult, OP.subtract)
                    sd = gp.tile([L, 1], f32, tag="sd")
                    nc.scalar.activation(sd[:], var[:], AF.Sqrt, bias=epsc_s[:, 0:1])
                    rstd = gp.tile([L, 1], f32, tag="rstd")
                    nc.vector.reciprocal(rstd[:], sd[:])
                    t1 = gp.tile([L, D], f32, tag="t1")
                    nc.vector.tensor_scalar(t1[:], zr[:], nm[:, 0:1],
                                            rstd[:, 0:1], OP.add, OP.mult)
                    t2 = gp.tile([L, D], f32, tag="t2")
                    nc.vector.tensor_mul(t2[:], t1[:], lng_s[:, li * D:(li + 1) * D])
                    if li < 2:
                        out_new = gp.tile([L, D], bf16, tag=f"out{li}")
                    else:
                        out_new = gp.tile([L, D], f32, tag="outf")
                    nc.vector.tensor_add(out_new[:], t2[:],
                                         lnb_s[:, li * D:(li + 1) * D])
                    out_cur = out_new

                nc.sync.dma_start(out_d[:], out_cur[:])

    nc.compile()
    return nc


def _get_nc():
    if "nc" not in _cache:
        _cache["nc"] = _build_bass()
    return _cache["nc"]


def make_in_maps(adj, inputs, w1, b1, w2, b2, w3, b3, gw, gb, lng, lnb):
    adj = np.asarray(adj, np.float32)
    x = np.asarray(inputs, np.float32)
    w1 = np.asarray(w1, np.float32)
    b1 = np.asarray(b1, np.float32)
    w2 = np.asarray(w2, np.float32)
    b2 = np.asarray(b2, np.float32)
    w3 = np.asarray(w3, np.float32)
    b3 = np.asarray(b3, np.float32)
    gw = np.asarray(gw, np.float32)
    gb = np.asarray(gb, np.float32)
    lng = np.asarray(lng, np.float32)
    lnb = np.asarray(lnb, np.float32)

    shared = {
        "eye": np.eye(L, dtype=np.float32),
        "idb": np.eye(L, dtype=BF),
        "onescol": np.ones((L, 1), BF),
        "onef": np.ones((1, 1), np.float32),
        "w1b": w1.astype(BF),
        "w2b": w2.astype(BF),
        "w3b": w3.reshape(D // 2, 1).astype(BF),
        "b1c": b1.reshape(D, 1),
        "b2c": b2.reshape(D // 2, 1),
        "b3bc": np.full((L, 1), b3[0], np.float32),
        "epsc": np.full((L, 1), 1e-5, np.float32),
        "gwb": gw.astype(BF),
        "gb2b": (2.0 * gb).reshape(3, 1, D).astype(BF),
        "onesb": np.ones((1, L), BF),
        "lngb": np.ascontiguousarray(
            np.broadcast_to(lng[:, None, :], (3, L, D))).astype(BF),
        "lnbb": np.ascontiguousarray(
            np.broadcast_to(lnb[:, None, :], (3, L, D))).astype(BF),
    }
    in_maps = []
    for c in range(B):
        m = dict(shared)
        m["xb"] = x[c].astype(BF)
        m["xTb"] = np.ascontiguousarray(x[c].T).astype(BF)
        m["adjT"] = np.ascontiguousarray(adj[c].T)
        in_maps.append(m)
    return in_maps


def kernel(**inputs):
    from concourse.bass_utils import run_bass_kernel_spmd

    in_maps = make_in_maps(**inputs)
    nc = _get_nc()
    res = run_bass_kernel_spmd(nc, in_maps, core_ids=list(range(B)))
    out = np.stack([res.results[c]["out"] for c in range(B)])
    mask = np.zeros((B, L, 1), bool)
    return out, mask
